# revision 12
# baseline (speedup 1.0000x reference)
"""Discretized-mixture NLL loss kernel for Trainium2 (Bass/Tile), 8-core data parallel.

Math (per pixel, per channel c, mixtures m=0..9), matching the reference:
    xhat = W @ px + b            (1x1 conv, 90 outputs = [pi(30) | mu(30) | ls(30)])
    sigma = exp(8*tanh(ls/8));  s2 = (1/sigma)/sqrt(2) = exp(-8*tanh(ls/8) + ln(1/sqrt2))
    D = mu - xe                  (xe folded into the conv via an extra K=4 matmul)
    dcdf = 0.5*(erf((D+d)*s2) - erf((D-d)*s2))          [erf odd]
    num  = sum_m exp(l_m)*dcdf_m ;  den = sum_m exp(l_m)
    nll  = log(den) - log(num + 1e-8*den)

v4 layout (per core, 16384 px = 16 supertiles of 1024). Flat software pipeline:
phase2 trails phase1 by `delay` supertiles and the erf->dlt->qd chain is
emitted at erf time (group end), so the PE reaches phase2 with its inputs
already resolved. Compute-engine AP bases obey the SB-SB equal-base rule.
  - psum ps [128, 1024] per supertile (2 banks, bufs=2): rows 0..31 D,
    32..63 ls, 64..95 pi, 96..127 tanh-out. fp32r matmuls: 4 K-chunks of
    128 from pair-sized xt loads + a K=4 chunk carrying (-xe, bias).
  - ScalarE: tanh (psum in-place); merged Exp -> s2g pair tile [128,1024]
    laid out [g0|s2_0|g1|s2_1] (out base 64*(s%2)). Erf over 4-supertile
    hi/lo stacks [128, 1024] (rows 32*(s%4)).
  - dlt = Elo - Ehi reads both erf tiles at base 32*(s%4) (equal), writes a
    [96,1024] tile at base 64*(s%2); qd = g*dlt then has in0/in1 both at
    base 64*(s%2) and overwrites the dead s2 rows -> [g|qd] per pair.
    dlt/qd alternate Vector/GpSimd and are emitted at group end.
  - PE reduction per pair: block-diagonal l1p [K=128, M=32] maps the two
    64-row [g|qd] blocks to output rows 16q'+v -> ONE N=512 matmul per
    supertile into ps2 [32, 1024] (2 banks, bufs=2).
  - Drain per pair: copy ps2 -> sc [32,1024] (alternating Scalar/Vector),
    4 respread DMAs -> packed (row 4v+g, col sub*128+p).
  - Tail: half A = first S-2 supertiles, half B = last 2: Ln[32, *] x2,
    nll = ln_d - ln_n (into ln_n), DMA rows 0..11 out. A overlaps B's
    compute; B's tail is tiny.
"""

import numpy as np

WIDTH = 512
C_IMG = 3
N_MIX = 10
SIZE = 64
STD = 127.5
EPS = 1e-8
DELTA = 1.0 / STD / 2.0
LOG_INV_SQRT2 = -0.34657359027997264
N_CORES = 8
SUP_W = 1024          # pixels per supertile
SUB_W = 512           # matmul moving-dim tile
QD = 4                # supertiles per erf batch


def make_consts(W, b):
    """Host-side prep of the small constant tensors (32-padded blocks)."""
    W = np.asarray(W, np.float32)
    b = np.asarray(b, np.float32)
    # lhsT column blocks: [mu(30)+2 | ls(30)+2 | pi(30)+2]; intra-block row 3m+c
    Wp = np.zeros((96, WIDTH), np.float32)
    bp = np.zeros(96, np.float32)
    Wp[0:30], bp[0:30] = W[30:60], b[30:60]     # mu
    Wp[32:62], bp[32:62] = W[60:90], b[60:90]   # logsigma
    Wp[64:94], bp[64:94] = W[0:30], b[0:30]     # pi logits
    wt = np.ascontiguousarray(Wp.T)             # [512, 96]
    bx = np.zeros((4, 96), np.float32)          # K=4 rows: (xe0, xe1, xe2, ones)
    for r in range(30):
        bx[r % 3, r] = -1.0                     # D rows get -xe_c
    bx[3, :] = bp                               # ones row carries the conv bias
    # block-diagonal pair reduction over s2g = [g0|qd0|g1|qd1]; M=32 cols,
    # block q' -> cols 16q'+v: v=0..2 num_c, 3..7 dummy, 8..10 den_c, 11..15 dummy
    l1p = np.zeros((128, 32), np.float32)
    for q in range(2):
        for r in range(30):
            c = r % 3
            l1p[64 * q + r, 16 * q + c] = EPS           # eps*den -> num column
            l1p[64 * q + r, 16 * q + 8 + c] = 1.0       # den
            l1p[64 * q + 32 + r, 16 * q + c] = 0.5      # +0.5*qd -> num
        for v in list(range(3, 8)) + list(range(11, 16)):
            l1p[64 * q + 0:64 * q + 30, 16 * q + v] = 1.0   # dummies keep Ln finite
    scb = np.zeros((64, 2), np.float32)         # merged-exp (scale, bias) rows
    scb[0:32, 0] = 1.0                          # pi rows: exp(x)
    scb[32:64, 0] = -8.0                        # tanh rows: exp(-8*t + ln(1/sqrt2))
    scb[32:64, 1] = LOG_INV_SQRT2
    return wt, bx, l1p, scb


def build_nc(n_batch=4, use_f32r=True, delay=4):
    """Build the single-core Bass program (same NEFF runs SPMD on all cores)."""
    from contextlib import ExitStack

    import concourse.bacc as bacc
    import concourse.mybir as mybir
    import concourse.tile as tile
    from concourse.tile import add_dep_helper

    f32 = mybir.dt.float32
    f32r = mybir.dt.float32r
    ALU = mybir.AluOpType
    ACT = mybir.ActivationFunctionType

    def mm_cast(ap):
        return ap.bitcast(f32r) if use_f32r else ap

    sup_per_batch = (SIZE * SIZE) // SUP_W
    S = n_batch * sup_per_batch                 # supertiles per core
    n_sub = S * (SUP_W // SUB_W)                # total subtiles
    assert S % QD == 0 and delay >= 3 and delay % 2 == 0
    n_subA = (S - 2) * 2                        # tail half A subtiles
    n_subB = 4

    nc = bacc.Bacc("TRN2", target_bir_lowering=False, debug=False)
    pz = nc.dram_tensor("pz", [n_batch, WIDTH, SIZE * SIZE], f32, kind="ExternalInput").ap()
    x4 = nc.dram_tensor("x4", [S, 4, SUP_W], f32, kind="ExternalInput").ap()
    wt = nc.dram_tensor("wt", [WIDTH, 96], f32, kind="ExternalInput").ap()
    bx = nc.dram_tensor("bx", [4, 96], f32, kind="ExternalInput").ap()
    l1 = nc.dram_tensor("l1", [128, 32], f32, kind="ExternalInput").ap()
    scb = nc.dram_tensor("scb", [64, 2], f32, kind="ExternalInput").ap()
    out = nc.dram_tensor("out", [12, 128 * n_sub], f32, kind="ExternalOutput").ap()

    with tile.TileContext(nc) as tc, ExitStack() as ctx:
        const_pool = ctx.enter_context(tc.tile_pool(name="const", bufs=1))
        xq_pool = ctx.enter_context(tc.tile_pool(name="xq", bufs=3))
        xt_pool = ctx.enter_context(tc.tile_pool(name="xt", bufs=4))
        s2g_pool = ctx.enter_context(tc.tile_pool(name="s2g", bufs=delay // 2 + 2))
        hl_pool = ctx.enter_context(tc.tile_pool(name="hl", bufs=2))
        e_pool = ctx.enter_context(tc.tile_pool(name="e", bufs=2))
        dlt_pool = ctx.enter_context(tc.tile_pool(name="dlt", bufs=5))
        sc_pool = ctx.enter_context(tc.tile_pool(name="sc", bufs=3))
        tail_pool = ctx.enter_context(tc.tile_pool(name="tail", bufs=1))
        ps_pool = ctx.enter_context(tc.tile_pool(name="ps", bufs=2, space="PSUM"))
        ps2_pool = ctx.enter_context(tc.tile_pool(name="ps2", bufs=2, space="PSUM"))

        # --- constants ---
        wt_sb = const_pool.tile([128, 4 * 96], f32)
        nc.sync.dma_start(
            wt_sb[:].rearrange("i (k o) -> i k o", o=96).bitcast(f32r),
            wt.rearrange("(k i) o -> i k o", i=128).bitcast(f32r),
        )
        bx_sb = const_pool.tile([4, 96], f32)
        nc.sync.dma_start(bx_sb[:].bitcast(f32r), bx.bitcast(f32r))
        l1_sb = const_pool.tile([128, 32], f32)
        nc.sync.dma_start(l1_sb[:].bitcast(f32r), l1.bitcast(f32r))
        scb_sb = const_pool.tile([64, 2], f32)
        nc.sync.dma_start(scb_sb[:], scb)

        packed = [
            tail_pool.tile([64, 128 * n_subA], f32, tag="packedA", name="packedA"),
            tail_pool.tile([64, 128 * n_subB], f32, tag="packedB", name="packedB"),
        ]

        # ACT table-set ordering chain (tanh/exp/erf/ln; Copy is in every set)
        act_chain = []

        def chain(inst):
            if act_chain:
                add_dep_helper(inst.ins, act_chain[-1].ins, sync=False,
                               reason="act table-set batching")
            act_chain.append(inst)
            return inst

        def load_pair(p):
            """DMA the two-supertile input block for pair p."""
            xq_t = xq_pool.tile([4, 2 * SUP_W], f32, tag="xq")
            nc.sync.dma_start(
                xq_t[:].rearrange("c (m w) -> c m w", m=2).bitcast(f32r),
                x4[2 * p:2 * p + 2].rearrange("m c w -> c m w").bitcast(f32r),
            )
            xts = []
            b, h2 = divmod(p, sup_per_batch // 2)
            for k in range(4):
                xt_t = xt_pool.tile([128, 2 * SUP_W], f32, tag="xt")
                nc.sync.dma_start(
                    xt_t[:].bitcast(f32r),
                    pz[b, 128 * k:128 * (k + 1),
                       2 * SUP_W * h2:2 * SUP_W * (h2 + 1)].bitcast(f32r),
                )
                xts.append(xt_t)
            return xq_t, xts

        def phase1(s, pair_in, s2g_t, hi_t, lo_t):
            xq_t, xts = pair_in
            m = s % 2
            hb = 32 * (s % QD)
            gb = 64 * m
            ps = ps_pool.tile([128, SUP_W], f32, tag="ps")
            for k in range(4):
                for t in range(SUP_W // SUB_W):
                    sl = slice(SUB_W * t, SUB_W * (t + 1))
                    sl_in = slice(SUP_W * m + SUB_W * t, SUP_W * m + SUB_W * (t + 1))
                    nc.tensor.matmul(
                        ps[0:96, sl], mm_cast(wt_sb[:, 96 * k:96 * (k + 1)]),
                        mm_cast(xts[k][:, sl_in]), start=(k == 0), stop=False,
                    )
            for t in range(SUP_W // SUB_W):
                sl = slice(SUB_W * t, SUB_W * (t + 1))
                sl_in = slice(SUP_W * m + SUB_W * t, SUP_W * m + SUB_W * (t + 1))
                nc.tensor.matmul(ps[0:96, sl], mm_cast(bx_sb[:]),
                                 mm_cast(xq_t[:, sl_in]), start=False, stop=True)
            chain(nc.scalar.activation(ps[96:128, :], ps[32:64, :], ACT.Tanh, scale=0.125))
            chain(nc.scalar.activation(
                s2g_t[gb:gb + 64, :].bitcast(f32r), ps[64:128, :], ACT.Exp,
                bias=scb_sb[:, 1:2], scale=scb_sb[:, 0:1],
            ))
            # hi' = (D - delta)*s2 ; lo' = (D + delta)*s2
            nc.vector.scalar_tensor_tensor(
                hi_t[hb:hb + 32, :], ps[0:32, :], DELTA,
                s2g_t[gb + 32:gb + 64, :], ALU.subtract, ALU.mult,
            )
            nc.vector.scalar_tensor_tensor(
                lo_t[hb:hb + 32, :], ps[0:32, :], DELTA,
                s2g_t[gb + 32:gb + 64, :], ALU.add, ALU.mult,
            )

        def emit_qd(s, s2g_t, ehi_t, elo_t):
            """dlt = Elo - Ehi, qd = g*dlt (overwrites s2 rows). All TT input
            pairs share a base partition: dlt reads at 32*(s%4), qd at 64*(s%2)."""
            hb = 32 * (s % QD)
            gb = 64 * (s % 2)
            eng0 = nc.gpsimd if s % 2 == 0 else nc.vector
            eng1 = nc.vector if s % 2 == 0 else nc.gpsimd
            dlt_t = dlt_pool.tile([96, SUP_W], f32, tag="dlt")
            eng0.tensor_tensor(dlt_t[gb:gb + 32, :], elo_t[hb:hb + 32, :],
                               ehi_t[hb:hb + 32, :], ALU.subtract)
            eng1.tensor_tensor(
                s2g_t[gb + 32:gb + 64, :].bitcast(f32r), s2g_t[gb:gb + 32, :],
                dlt_t[gb:gb + 32, :], ALU.mult,
            )

        def sub_dst(sub):
            if sub < n_subA:
                return packed[0], sub
            return packed[1], sub - n_subA

        def phase2_pair(p, s2g_t):
            ps2 = ps2_pool.tile([32, SUP_W], f32, tag="ps2")
            for t in range(SUP_W // SUB_W):
                sl = slice(SUB_W * t, SUB_W * (t + 1))
                nc.tensor.matmul(ps2[0:32, sl], mm_cast(l1_sb[:]),
                                 mm_cast(s2g_t[:, sl]), start=True, stop=True)
            sc_t = sc_pool.tile([32, SUP_W], f32, tag="sc")
            if p % 2 == 0:
                nc.scalar.copy(sc_t[:], ps2[:])
            else:
                nc.vector.tensor_scalar_add(sc_t[:], ps2[:], 0.0)
            for q in range(2):
                for t in range(SUP_W // SUB_W):
                    sub = 4 * p + 2 * q + t
                    dst, subh = sub_dst(sub)
                    nc.sync.dma_start(
                        dst[:, 128 * subh:128 * (subh + 1)],
                        sc_t[16 * q:16 * q + 16, SUB_W * t:SUB_W * (t + 1)]
                        .rearrange("v (g p) -> v g p", p=128),
                    )

        def emit_tail(half, nsub, col0):
            w = 128 * nsub
            ln_n = tail_pool.tile([32, w], f32, tag=f"ln_n{half}", name=f"ln_n{half}", bufs=1)
            ln_d = tail_pool.tile([32, w], f32, tag=f"ln_d{half}", name=f"ln_d{half}", bufs=1)
            chain(nc.scalar.activation(ln_n[:], packed[half][0:32, :], ACT.Ln))
            chain(nc.scalar.activation(ln_d[:], packed[half][32:64, :], ACT.Ln))
            # nll = ln_d - ln_n, written back into ln_n
            nc.vector.tensor_tensor(ln_n[:], ln_d[:], ln_n[:], ALU.subtract)
            nc.sync.dma_start(out[0:12, col0:col0 + w], ln_n[0:12, :])

        # --- flat software pipeline ---
        s2gs = {}       # pair -> s2g tile
        pair_ins = {}   # pair -> (xq_t, xts)
        erfs = {}       # group -> (ehi, elo)
        hi_t = lo_t = None
        for i in range(S + delay):
            if i < S:
                if i % 2 == 0:
                    pair_ins[i // 2] = load_pair(i // 2)
                    s2gs[i // 2] = s2g_pool.tile([128, SUP_W], f32, tag="s2g",
                                                 name=f"s2g{i // 2}")
                if i % QD == 0:
                    hi_t = hl_pool.tile([128, SUP_W], f32, tag="hi")
                    lo_t = hl_pool.tile([128, SUP_W], f32, tag="lo")
                phase1(i, pair_ins[i // 2], s2gs[i // 2], hi_t, lo_t)
                if i % QD == QD - 1:
                    ehi_t = e_pool.tile([128, SUP_W], f32, tag="ehi")
                    elo_t = e_pool.tile([128, SUP_W], f32, tag="elo")
                    chain(nc.scalar.activation(ehi_t[:], hi_t[:], ACT.Erf))
                    chain(nc.scalar.activation(elo_t[:], lo_t[:], ACT.Erf))
                    for s in range(i - QD + 1, i + 1):
                        emit_qd(s, s2gs[s // 2], ehi_t, elo_t)
            j = i - delay
            if j >= 0 and j % 2 == 1:
                p = j // 2
                phase2_pair(p, s2gs.pop(p))
                pair_ins.pop(p, None)
                if j == S - 3:
                    emit_tail(0, n_subA, 0)
                elif j == S - 1:
                    emit_tail(1, n_subB, 128 * n_subA)

    nc.compile()
    return nc


def prep_core_inputs(px_z_shard, x_shard, consts):
    """px_z_shard [nb, 512, 64, 64], x_shard [nb, 64, 64, 3] -> input map."""
    wt, bx, l1p, scb = consts
    nb = px_z_shard.shape[0]
    S = nb * (SIZE * SIZE) // SUP_W
    pzs = np.ascontiguousarray(px_z_shard.reshape(nb, WIDTH, SIZE * SIZE))
    xf = x_shard.reshape(S, SUP_W, C_IMG)
    x4 = np.ones((S, 4, SUP_W), np.float32)
    x4[:, 0:3, :] = xf.transpose(0, 2, 1)
    return {
        "pz": pzs, "x4": np.ascontiguousarray(x4), "wt": wt, "bx": bx,
        "l1": l1p, "scb": scb,
    }


def gather_core_output(o, nb):
    """o [12, 128*n_sub] (row 4c+g', col sub*128+p') -> [nb, 64, 64, 3]."""
    n_sub = nb * (SIZE * SIZE) // SUB_W
    return (
        o.reshape(C_IMG, 4, n_sub, 128).transpose(2, 1, 3, 0)
        .reshape(nb, SIZE, SIZE, C_IMG)
    )


_NC_CACHE = {}


def kernel(px_z, x, W, b):
    from concourse.bass_utils import run_bass_kernel_spmd

    px_z = np.asarray(px_z, np.float32)
    x = np.asarray(x, np.float32)
    B = px_z.shape[0]
    nb = B // N_CORES
    consts = make_consts(W, b)
    key = (nb,)
    if key not in _NC_CACHE:
        _NC_CACHE[key] = build_nc(n_batch=nb)
    nc = _NC_CACHE[key]
    in_maps = [
        prep_core_inputs(px_z[nb * i:nb * (i + 1)], x[nb * i:nb * (i + 1)], consts)
        for i in range(N_CORES)
    ]
    res = run_bass_kernel_spmd(nc, in_maps, core_ids=list(range(N_CORES)))
    outs = [gather_core_output(res.results[i]["out"], nb) for i in range(N_CORES)]
    return np.concatenate(outs, 0)
